# revision 1
# baseline (speedup 1.0000x reference)
"""Block-sparse self-attention (DeepSpeed "fixed" layout) on 8 trn2 cores.

Problem: B=2, H=16, S=2048, D=64 fp32. Mask (identical for every head,
since numverts=1): each 64-wide diagonal window is dense, plus every 4th
16-col block ("stripe") is attended by all queries. Per 64-row query
window the attended key set = its 64 window cols + 512 stripe cols,
overlapping by 16 -> 560 distinct keys.

Sharding: 32 (b,h) pairs -> 4 per core (batch+head parallel).

Host prep per pair (pure layout + dtype cast):
  qT  [64, 2048]: Q^T.   kT [64, 2048]: K^T with columns reordered to
      [512 stripe cols | 32 windows x 48 non-stripe cols].
  vva [2048, 65]: V rows in the same reorder + a ones column (rides the
      PV matmul; lands the softmax denominator L in O' row 64).

On chip per pair (all matmul operands at base partition 0 — alternating
weight-load base partitions between instructions faults the device):
  S^T[k,q] = matmul(lhsT=K^T chunk, rhs=Q^T)          (PSUM fp32)
  P = exp(0.125 * S^T)  on ACT, fp16 -> SBUF           (scale fused)
  O'^T[65,q] += matmul(lhsT=V_aug chunk, rhs=P chunk)  (PSUM fp32)
  r = 1/L (row 64), broadcast across partitions, O = O'[0:64] * r
  out[pair] = O^T [64, 2048] fp32; host transposes back.
"""

import numpy as np

B, H, S, D = 2, 16, 2048, 64
NPAIRS = B * H
NCORES = 8
P_PER_CORE = NPAIRS // NCORES  # 4
NCH = 4        # stripe k-chunks of 128
NW = S // 64   # 32 windows
SCALE = float(D) ** -0.5


def _reorder_idx():
    blocks = np.arange(S // 16)
    stripe = blocks[blocks % 4 == 3]
    rest = blocks[blocks % 4 != 3]
    cols = np.arange(S).reshape(-1, 16)
    return np.concatenate([cols[stripe].ravel(), cols[rest].ravel()])


_REORDER = _reorder_idx()

_CACHE = {}


def _build(dt_in_name="float16", npairs=P_PER_CORE):
    from contextlib import ExitStack
    import concourse.bacc as bacc
    import concourse.tile as tile
    from concourse import mybir

    dt_in = getattr(mybir.dt, dt_in_name)
    f32 = mybir.dt.float32
    EXP = mybir.ActivationFunctionType.Exp

    nc = bacc.Bacc("TRN2", target_bir_lowering=False, debug=False,
                   num_devices=NCORES)
    qT = nc.dram_tensor("qT", [P_PER_CORE, 64, S], dt_in,
                        kind="ExternalInput").ap()
    kT = nc.dram_tensor("kT", [P_PER_CORE, 64, S], dt_in,
                        kind="ExternalInput").ap()
    vva = nc.dram_tensor("vva", [P_PER_CORE, S, 65], dt_in,
                         kind="ExternalInput").ap()
    out = nc.dram_tensor("out", [P_PER_CORE, 64, S], f32,
                         kind="ExternalOutput").ap()

    with tile.TileContext(nc) as tc, ExitStack() as ctx:
        qk_pool = ctx.enter_context(tc.tile_pool(name="qk", bufs=2))
        v_pool = ctx.enter_context(tc.tile_pool(name="v", bufs=2))
        p_pool = ctx.enter_context(tc.tile_pool(name="p", bufs=2))
        n_pool = ctx.enter_context(tc.tile_pool(name="n", bufs=2))
        s_pool = ctx.enter_context(tc.tile_pool(name="s", bufs=2, space="PSUM"))
        o_pool = ctx.enter_context(tc.tile_pool(name="o", bufs=1, space="PSUM"))

        for p in range(npairs):
            qt = qk_pool.tile([64, S], dt_in, tag="q")
            nc.sync.dma_start(out=qt, in_=qT[p])
            kt = qk_pool.tile([64, S], dt_in, tag="k")
            nc.sync.dma_start(out=kt, in_=kT[p])
            vs = v_pool.tile([128, NCH, 65], dt_in, tag="vs")
            nc.sync.dma_start(
                out=vs, in_=vva[p, 0:512].rearrange("(c r) d -> r c d", r=128))
            vw = v_pool.tile([48, NW * 65], dt_in, tag="vw")
            vw3 = vw.rearrange("j (w d) -> j w d", d=65)
            nc.sync.dma_start(
                out=vw3, in_=vva[p, 512:S].rearrange("(w j) d -> j w d", j=48))

            ps = p_pool.tile([128, NCH, S], dt_in, tag="ps")
            pw = p_pool.tile([48, NW * 64], dt_in, tag="pw")

            # stripe scores + exp, in [128, 1024] PSUM tiles (2 banks each)
            for c in range(NCH):
                for h in range(2):
                    st = s_pool.tile([128, 1024], f32, tag="s")
                    for g in range(2):
                        q0 = h * 1024 + g * 512
                        nc.tensor.matmul(
                            out=st[:, g * 512:(g + 1) * 512],
                            lhsT=kt[:, c * 128:(c + 1) * 128],
                            rhs=qt[:, q0:q0 + 512],
                            start=True, stop=True)
                    nc.scalar.activation(
                        out=ps[:, c, h * 1024:(h + 1) * 1024], in_=st,
                        func=EXP, scale=SCALE)

            # window scores: window w -> partitions 0:48, free offset 64*(w%16)
            for h in range(2):
                sw = s_pool.tile([48, 1024], f32, tag="s")
                for w in range(h * 16, h * 16 + 16):
                    fo = (w - h * 16) * 64
                    nc.tensor.matmul(
                        out=sw[:, fo:fo + 64],
                        lhsT=kt[:, 512 + 48 * w:512 + 48 * w + 48],
                        rhs=qt[:, 64 * w:64 * w + 64],
                        start=True, stop=True)
                nc.scalar.activation(
                    out=pw[:, h * 1024:(h + 1) * 1024], in_=sw,
                    func=EXP, scale=SCALE)

            # PV: accumulate O'^T [65, q] over 4 stripe chunks + windows
            ov = o_pool.tile([65, S], f32, tag="o")
            for g in range(4):
                q0 = g * 512
                for c in range(NCH):
                    nc.tensor.matmul(
                        out=ov[:, q0:q0 + 512],
                        lhsT=vs[:, c, :],
                        rhs=ps[:, c, q0:q0 + 512],
                        start=(c == 0), stop=False, skip_group_check=True)
            for w in range(NW):
                nc.tensor.matmul(
                    out=ov[:, 64 * w:64 * w + 64],
                    lhsT=vw[:, 65 * w:65 * w + 65],
                    rhs=pw[:, 64 * w:64 * w + 64],
                    start=False, stop=(w == NW - 1), skip_group_check=True)

            # normalize: r = 1/L, broadcast, multiply. The L row sits at
            # PSUM partition 64; custom-DVE ops misread nonzero base
            # partitions on HW, so: native copy to SBUF@64, DMA to
            # partition 0, reciprocal there, then broadcast.
            lt = n_pool.tile([65, S], f32, tag="l")
            nc.vector.tensor_copy(lt[64:65], ov[64:65, :])
            rt = n_pool.tile([1, S], f32, tag="r")
            nc.sync.dma_start(out=rt, in_=lt[64:65])
            rr = n_pool.tile([1, S], f32, tag="rr")
            nc.vector.reciprocal_approx_fast(out=rr, in_=rt)
            rb = n_pool.tile([64, S], f32, tag="rb")
            nc.gpsimd.partition_broadcast(rb, rr[0:1])
            ob = n_pool.tile([64, S], f32, tag="ob")
            nc.vector.tensor_mul(out=ob, in0=ov[0:64, :], in1=rb)
            nc.sync.dma_start(out=out[p], in_=ob)

    nc.compile()
    return nc


def _get_nc(dt_in_name="float16"):
    if dt_in_name not in _CACHE:
        _CACHE[dt_in_name] = _build(dt_in_name)
    return _CACHE[dt_in_name]


def _prep_inputs(query, key, value, np_dt):
    q = np.asarray(query).reshape(NPAIRS, S, D)
    k = np.asarray(key).reshape(NPAIRS, S, D)
    v = np.asarray(value).reshape(NPAIRS, S, D)
    kr = k[:, _REORDER, :]
    vr = v[:, _REORDER, :]
    qT = np.ascontiguousarray(q.transpose(0, 2, 1)).astype(np_dt)
    kT = np.ascontiguousarray(kr.transpose(0, 2, 1)).astype(np_dt)
    vva = np.concatenate(
        [vr, np.ones((NPAIRS, S, 1), vr.dtype)], axis=2).astype(np_dt)
    in_maps = []
    for core in range(NCORES):
        sl = slice(core * P_PER_CORE, (core + 1) * P_PER_CORE)
        in_maps.append({"qT": np.ascontiguousarray(qT[sl]),
                        "kT": np.ascontiguousarray(kT[sl]),
                        "vva": np.ascontiguousarray(vva[sl])})
    return in_maps


def _run(query, key, value, dt_in_name="float16", trace=False):
    from concourse.bass_utils import run_bass_kernel_spmd
    nc = _get_nc(dt_in_name)
    in_maps = _prep_inputs(query, key, value, np.float16
                           if dt_in_name == "float16" else np.float32)
    res = run_bass_kernel_spmd(nc, in_maps, list(range(NCORES)), trace=trace)
    o = np.concatenate([res.results[i]["out"] for i in range(NCORES)], axis=0)
    full = o.transpose(0, 2, 1).reshape(B, H, S, D).astype(np.float32)
    return full, res


def kernel(query, key, value):
    full, _ = _run(np.asarray(query), np.asarray(key), np.asarray(value))
    return full



# revision 9
# speedup vs baseline: 1.3405x; 1.3405x over previous
"""Block-sparse self-attention (DeepSpeed "fixed" layout) on 8 trn2 cores, v2.

Problem: B=2, H=16, S=2048, D=64 fp32. Mask (identical for every head):
each 64-wide diagonal window is dense, plus every 4th 16-col block
("stripe") is attended by all queries.

Sharding: 32 (b,h) pairs -> 4 per core. All tensors fp16 on device,
natural sequence order. Output fp16 [64, S] per pair; host casts back.

v2 structure per pair, two 1024-query halves pipelined through PSUM:
  stripe QK: lhsT = gathered stripe cols of K^T, 8 MMs of N=512/half.
  window QK: 2 windows packed per MM ([64,128] lhsT, N=128), 8/half.
    exp with per-partition bias -30 kills the stripe-overlap keys
    (cols 48:64 of each window); cross-window garbage quadrants never
    get exp'd (strided ACT) and stay 0 in the static pw tile.
  exp split ACT/DVE; DVE chunks use a one-op Schraudolph:
    int16(a*s + b) bitcast to fp16 ~= exp(0.125*s) within ~2.5% rms;
    the error largely cancels in the softmax ratio.
  PV lhsT = [ones(64) | V(64)]: PSUM out [128, 1024] has L replicated
    on rows 0:64 (free PE broadcast of the denominator) and O'^T on
    rows 64:128. Normalize = reciprocal_approx_fast on [64,1024]@p0,
    DMA partition-shift to p64:128, one tensor_mul -> fp16 out. No
    1-lane ops, no partition_broadcast.
"""

import numpy as np

B, H, S, D = 2, 16, 2048, 64
NPAIRS = B * H
NCORES = 8
P_PER_CORE = NPAIRS // NCORES  # 4
NCH = 4          # stripe key chunks of 128
HALF = 1024      # queries per half
SCALE = float(D) ** -0.5

# Schraudolph exp(0.125*s) in fp16 bits: int16(a*s + b) ~= fp16 bits of
# exp(0.125*s). a = 0.125*log2(e)*1024; b = 15*1024 - 1024*c with
# c = 0.054799 minimizing RMS relative error of (1+f-c)/2^f.
EXP_A = 184.6649652337873
EXP_B = 15303.886
NEG = -30.0

# engine per stripe chunk's exp (windows stay on ACT)
EXP_ENGINE = {0: "act", 1: "act", 2: "act", 3: "act"}

_CACHE = {}


def _build():
    from contextlib import ExitStack
    import concourse.bacc as bacc
    import concourse.tile as tile
    from concourse import mybir

    f16 = mybir.dt.float16
    f32 = mybir.dt.float32
    i16 = mybir.dt.int16
    EXP = mybir.ActivationFunctionType.Exp
    MUL = mybir.AluOpType.mult
    ADD = mybir.AluOpType.add

    nc = bacc.Bacc("TRN2", target_bir_lowering=False, debug=False,
                   num_devices=NCORES)
    qT = nc.dram_tensor("qT", [P_PER_CORE, 64, S], f16,
                        kind="ExternalInput").ap()
    kT = nc.dram_tensor("kT", [P_PER_CORE, 64, S], f16,
                        kind="ExternalInput").ap()
    vv = nc.dram_tensor("vv", [P_PER_CORE, S, 64], f16,
                        kind="ExternalInput").ap()
    vvs = nc.dram_tensor("vvs", [P_PER_CORE, 512, 64], f16,
                         kind="ExternalInput").ap()
    bmask = nc.dram_tensor("bmask", [128, 1], f32,
                           kind="ExternalInput").ap()
    out = nc.dram_tensor("out", [P_PER_CORE, 64, S], f16,
                         kind="ExternalOutput").ap()

    with tile.TileContext(nc) as tc, ExitStack() as ctx:
        io_pool = ctx.enter_context(tc.tile_pool(name="io", bufs=2))
        p_pool = ctx.enter_context(tc.tile_pool(name="p", bufs=2))
        st_pool = ctx.enter_context(tc.tile_pool(name="st", bufs=2,
                                                 space="PSUM"))
        w_pool = ctx.enter_context(tc.tile_pool(name="w", bufs=1,
                                                space="PSUM"))
        o_pool = ctx.enter_context(tc.tile_pool(name="o", bufs=1,
                                                space="PSUM"))
        n_pool = ctx.enter_context(tc.tile_pool(name="n", bufs=2))
        c_pool = ctx.enter_context(tc.tile_pool(name="c", bufs=1))

        # one-time: bias mask and static zeroed window-P tile
        bias = c_pool.tile([128, 1], f32, tag="bias")
        nc.sync.dma_start(out=bias, in_=bmask)
        pw = c_pool.tile([128, 16, 128], f16, tag="pw")
        nc.vector.memset(pw, 0.0)

        for p in range(P_PER_CORE):
            qt = io_pool.tile([64, S], f16, tag="q")
            nc.sync.dma_start(out=qt, in_=qT[p])
            kt = io_pool.tile([64, S], f16, tag="k")
            nc.sync.dma_start(out=kt, in_=kT[p])
            # V_aug = [ones 64 | V 64]; ones cols memset once per buffer
            vw = io_pool.tile([128, 16, 128], f16, tag="vw")
            vs = io_pool.tile([128, NCH, 128], f16, tag="vs")
            if p < 2:
                for g in range(16):
                    nc.vector.memset(vw[:, g, 0:64], 1.0)
                for c in range(NCH):
                    nc.vector.memset(vs[:, c, 0:64], 1.0)
            # window V: partition = row within 128-group, free = (group, d)
            nc.sync.dma_start(
                out=vw[:, :, 64:128],
                in_=vv[p].rearrange("(g r) d -> r g d", r=128))
            # stripe V rows (host-gathered), plain SBUF out AP
            nc.sync.dma_start(
                out=vs[:, :, 64:128],
                in_=vvs[p].rearrange("(c r) d -> r c d", r=128))
            # stripe K cols gathered contiguous (weights need 1 free dim)
            kts = io_pool.tile([64, 512], f16, tag="ks")
            nc.sync.dma_start(
                out=kts,
                in_=kT[p].rearrange("d (w r) -> d w r", r=64)[:, :, 48:64])

            for h in range(2):
                q0 = h * HALF
                ps = p_pool.tile([128, NCH, HALF], f16, tag="ps")
                psi = ps.bitcast(i16)

                # stripe scores
                for c in range(NCH):
                    stt = st_pool.tile([128, HALF], f32, tag="s")
                    for j in range(2):
                        nc.tensor.matmul(
                            out=stt[:, j * 512:(j + 1) * 512],
                            lhsT=kts[:, 128 * c:128 * (c + 1)],
                            rhs=qt[:, q0 + j * 512:q0 + (j + 1) * 512],
                            start=True, stop=True)
                    if EXP_ENGINE[c] == "act":
                        nc.scalar.activation(out=ps[:, c, :], in_=stt,
                                             func=EXP, scale=SCALE)
                    else:
                        nc.vector.tensor_scalar(
                            out=psi[:, c, :], in0=stt,
                            scalar1=EXP_A, scalar2=EXP_B, op0=MUL, op1=ADD)

                # window scores: 2 windows per MM, groups g = 8h+gl
                wt = w_pool.tile([128, 8, 128], f32, tag="w")
                for gl in range(8):
                    g = 8 * h + gl
                    nc.tensor.matmul(
                        out=wt[:, gl, :],
                        lhsT=kt[:, 128 * g:128 * (g + 1)],
                        rhs=qt[:, 128 * g:128 * (g + 1)],
                        start=True, stop=True)
                nc.scalar.activation(
                    out=pw[0:64, 8 * h:8 * h + 8, 0:64],
                    in_=wt[0:64, :, 0:64],
                    func=EXP, scale=SCALE, bias=bias[0:64])
                nc.scalar.activation(
                    out=pw[64:128, 8 * h:8 * h + 8, 64:128],
                    in_=wt[64:128, :, 64:128],
                    func=EXP, scale=SCALE, bias=bias[64:128])

                # PV into [128, 1024]: rows 0:64 = L replica, 64:128 = O'^T
                ov = o_pool.tile([128, HALF], f32, tag="o")
                nmm = 0
                last = 2 * NCH + 8
                for j in range(2):
                    for c in range(NCH):
                        nmm += 1
                        nc.tensor.matmul(
                            out=ov[:, j * 512:(j + 1) * 512],
                            lhsT=vs[:, c, :],
                            rhs=ps[:, c, j * 512:(j + 1) * 512],
                            start=(c == 0), stop=False,
                            skip_group_check=True)
                for gl in range(8):
                    g = 8 * h + gl
                    nmm += 1
                    nc.tensor.matmul(
                        out=ov[:, gl * 128:(gl + 1) * 128],
                        lhsT=vw[:, g, :],
                        rhs=pw[:, g, :],
                        start=False, stop=(nmm == last),
                        skip_group_check=True)

                # normalize: r = 1/L on partitions 0:64, shift to 64:128,
                # multiply O'^T rows in place of partitions 64:128
                lcp = n_pool.tile([64, HALF], f32, tag="lcp")
                nc.vector.tensor_copy(lcp, ov[0:64, :])
                rbi = n_pool.tile([64, HALF], f32, tag="rbi")
                nc.vector.reciprocal_approx_fast(out=rbi, in_=lcp)
                ob = n_pool.tile([64, HALF], f16, tag="ob")
                nc.vector.tensor_mul(out=ob, in0=ov[64:128, :], in1=rbi)
                nc.sync.dma_start(out=out[p][:, q0:q0 + HALF], in_=ob)

    nc.compile()
    return nc


def _get_nc():
    if "nc" not in _CACHE:
        _CACHE["nc"] = _build()
    return _CACHE["nc"]


def _prep_inputs(query, key, value):
    q = np.asarray(query).reshape(NPAIRS, S, D)
    k = np.asarray(key).reshape(NPAIRS, S, D)
    v = np.asarray(value).reshape(NPAIRS, S, D)
    qT = np.ascontiguousarray(q.transpose(0, 2, 1)).astype(np.float16)
    kT = np.ascontiguousarray(k.transpose(0, 2, 1)).astype(np.float16)
    vv = v.astype(np.float16)
    stripe = (np.arange(S) // 16) % 4 == 3
    vvs = np.ascontiguousarray(vv[:, stripe, :])
    bmask = np.zeros((128, 1), np.float32)
    bmask[48:64] = NEG
    bmask[112:128] = NEG
    in_maps = []
    for core in range(NCORES):
        sl = slice(core * P_PER_CORE, (core + 1) * P_PER_CORE)
        in_maps.append({"qT": np.ascontiguousarray(qT[sl]),
                        "kT": np.ascontiguousarray(kT[sl]),
                        "vv": np.ascontiguousarray(vv[sl]),
                        "vvs": np.ascontiguousarray(vvs[sl]),
                        "bmask": bmask})
    return in_maps


def _run(query, key, value, trace=False):
    from concourse.bass_utils import run_bass_kernel_spmd
    nc = _get_nc()
    in_maps = _prep_inputs(query, key, value)
    res = run_bass_kernel_spmd(nc, in_maps, list(range(NCORES)), trace=trace)
    o = np.concatenate([res.results[i]["out"] for i in range(NCORES)], axis=0)
    full = o.astype(np.float32).transpose(0, 2, 1).reshape(B, H, S, D)
    return full, res


def kernel(query, key, value):
    full, _ = _run(np.asarray(query), np.asarray(key), np.asarray(value))
    return full
